# revision 26
# baseline (speedup 1.0000x reference)
"""Bass/Tile TRN2 kernel for a tanh RNN layer (BasicRecurrentLayer).

  h = einsum('btf,fu->tbu', inputs, R)
  s_t = tanh(h_t + s_{t-1} @ W + bias);  out[t] = s_t        (T sequential steps)

Shapes: inputs [B=64, T=1024, F=128], R [F, U=256], W [U, U], bias [U], x0 [U].
Output: [T, B, U] float32.

Sharding: data-parallel over batch across 8 cores (8 rows each); R/W/bias/x0
replicated. The T recurrence is inherently serial, so the kernel is built
around minimizing the per-step critical path:

  - transposed state layout sT [U(2x128 part-chunks), B=8] kept entirely in SBUF
  - per step: psum[:,0:16] = I128 @ H_t  (injects h_t+bias, no state dep)
              psum[:, m*8:+8] += W[c,m-block].T @ sT_c   (4 matmuls, W stationary)
              sT_new = tanh(psum)  (single ACT instruction per step)
  - H precomputed in phase 1 (X@R with N=512 matmuls), chunked for pipelining
  - states accumulate in SBUF chunks of 64 steps, DMA'd out in 1MB blocks
"""

import numpy as np

import bass_rust
import concourse.bass as bass
import concourse.mybir as mybir
import concourse.tile as tile
from concourse.bass_utils import run_bass_kernel_spmd
from concourse.masks import make_identity

B, T, F, U = 64, 1024, 128, 256
NCORES = 8
BS = B // NCORES            # 8 batch rows per core
P = 128                     # SBUF partitions
MC = U // P                 # 2 state chunks of 128
CHUNK = 64                  # recurrence steps per state/H chunk buffer
FP32 = mybir.dt.float32


def build_nc(t_steps: int = T, use_f32r: bool = False,
             rec_f32r: bool | None = None) -> bass.Bass:
    """use_f32r: stream phase-1 matmul operands as float32r (same fp32 bits,
    single-pass PE mode, 4x faster at N>=256). rec_f32r: same for the
    recurrence operands (W/H/state); defaults to use_f32r."""
    if rec_f32r is None:
        rec_f32r = use_f32r
    assert t_steps % CHUNK == 0
    n_chunks = t_steps // CHUNK
    nc = bass.Bass()

    # float32r has identical fp32 bits/size (np dtype is float32) but streams
    # through the PE in single-pass reduced-precision mode (~tf32/bf16x2).
    # All producers of float32r buffers must emit float32r (BIR verifier).
    PDT = mybir.dt.float32r if use_f32r else FP32   # phase-1 operands
    MDT = mybir.dt.float32r if rec_f32r else FP32   # recurrence operands

    def mmap(ap):  # matmul-operand view (no-op now; tiles carry dtypes)
        return ap

    x_in = nc.declare_dram_parameter("x", [BS, t_steps, F], PDT, isOutput=False)
    w_in = nc.declare_dram_parameter("w", [U, U], MDT, isOutput=False)
    r_in = nc.declare_dram_parameter("r", [F, U], PDT, isOutput=False)
    b_in = nc.declare_dram_parameter("bias", [U], FP32, isOutput=False)
    x0_in = nc.declare_dram_parameter("x0", [U], FP32, isOutput=False)
    # out is declared with the recurrence dtype label so the state-chunk DMA
    # is cast-free; float32r's numpy dtype is plain float32 either way.
    out_d = nc.declare_dram_parameter("out", [t_steps, BS, U], MDT, isOutput=True)

    with tile.TileContext(nc) as tc:
        with (
            tc.tile_pool(name="sb", bufs=1) as sb,
            tc.tile_pool(name="ps_h", bufs=2, space="PSUM") as ps_h,
            tc.tile_pool(name="ps_rec", bufs=4, space="PSUM") as ps_rec,
        ):
            # ---- persistent SBUF buffers ----
            xt = sb.tile([P, t_steps * BS], MDT, tag="xt")       # X.T, col = t*BS+b
            w_sb = sb.tile([P, MC * U], MDT, tag="w")            # col = c*U + j
            r_sb = sb.tile([P, U], MDT, tag="r")
            ident = sb.tile([P, P], MDT, tag="ident")
            bias_sb = sb.tile([P, MC], FP32, tag="bias")         # col m: bias[m*128+p]
            x0_sb = sb.tile([P, MC], FP32, tag="x0")
            state0 = sb.tile([P, MC * BS], MDT, tag="state0")    # col = m*BS+b
            h_ch = [sb.tile([P, CHUNK * MC * BS], MDT, name=f"h{i}", tag=f"h{i}")
                    for i in range(n_chunks)]
            st_ch = [sb.tile([P, CHUNK * MC * BS], MDT, name=f"s{i}", tag=f"s{i}")
                     for i in range(n_chunks)]

            # ---- constant / input loads ----
            if MDT == FP32:
                make_identity(nc, ident[:])
            else:
                # gpsimd memset/affine_select fail the ISA check on float32r;
                # build the identity in fp32 and cast-copy on the DVE.
                ident_f32 = sb.tile([P, P], FP32, tag="ident_f32")
                make_identity(nc, ident_f32[:])
                nc.vector.tensor_copy(out=ident[:], in_=ident_f32[:])
            for c in range(MC):
                nc.sync.dma_start(out=w_sb[:, c * U:(c + 1) * U],
                                  in_=w_in[c * P:(c + 1) * P, :])
            nc.sync.dma_start(out=r_sb[:], in_=r_in[:])
            nc.sync.dma_start(out=bias_sb[:],
                              in_=b_in[:].rearrange("(m p) -> p m", m=MC))
            nc.sync.dma_start(out=x0_sb[:],
                              in_=x0_in[:].rearrange("(m p) -> p m", m=MC))
            for m in range(MC):
                nc.vector.tensor_copy(
                    out=state0[:, m * BS:(m + 1) * BS],
                    in_=x0_sb[:, m:m + 1].to_broadcast([P, BS]),
                )
            # observe the bias DMA's queue sem on the DVE (1 wait here) so
            # phase-1 tensor_scalar_adds only carry their psum (PE) wait —
            # walrus allows a single sync wait per DVE instruction.
            scratch = sb.tile([P, MC], FP32, tag="scratch")
            nc.vector.tensor_copy(out=scratch[:], in_=bias_sb[:])

            # inputs -> X.T in SBUF, one DMA per batch row (2D APs each side)
            xt_v = xt[:].rearrange("p (t b) -> p t b", b=BS)
            for b in range(BS):
                nc.sync.dma_start(
                    out=xt_v[:, :, b],
                    in_=x_in[b, :, :].rearrange("t f -> f t"),
                )

            # ---- PE observation ladder ----
            # walrus can attach only ONE sem wait to a Matmult (fp32
            # self-loading LDWEIGHTS wait table). The 8 input DMAs land on 8
            # different HW-DGE queue sems, so a phase-1 matmul reading XT
            # would need up to 8 waits. These dummy matmuls observe one
            # queue sem each so later PE instructions need no input waits.
            junk_ps = ps_h.tile([P, 2], FP32, tag="junk", bufs=1)
            for k in range(BS):
                nc.tensor.matmul(
                    out=junk_ps[0:1, 0:1],
                    lhsT=xt[:, k:k + 1],
                    rhs=xt[:, k:k + 1],
                    start=True, stop=True,
                )

            # ---- phase 1: H.T = R.T @ X.T (+bias), chunked to pipeline ----
            # psum [128, CHUNK*BS] per (chunk, m); cols (t_local, b)
            for i in range(n_chunks):
                hv = h_ch[i][:].rearrange("p (t m b) -> p t m b", m=MC, b=BS)
                for m in range(MC):
                    if i > 0:
                        # the psum slot for (i, m) was last read by the DVE
                        # copy of (i-1, m); observe that DVE tick with a
                        # 1-wait dummy so the real matmul below only carries
                        # its PE (prior-matmul completion) wait.
                        nc.tensor.matmul(
                            out=junk_ps[0:1, 0:1],
                            lhsT=h_ch[i - 1][:, m * BS:m * BS + 1],
                            rhs=h_ch[i - 1][:, m * BS:m * BS + 1],
                            start=True, stop=True,
                        )
                    ph = ps_h.tile([P, CHUNK * BS], FP32)
                    nc.tensor.matmul(
                        out=ph[:],
                        lhsT=mmap(r_sb[:, m * P:(m + 1) * P]),
                        rhs=mmap(xt[:, i * CHUNK * BS:(i + 1) * CHUNK * BS]),
                        start=True, stop=True,
                    )
                    nc.vector.tensor_scalar_add(
                        out=hv[:, :, m, :],
                        in0=ph[:].rearrange("p (t b) -> p t b", b=BS),
                        scalar1=bias_sb[:, m:m + 1],
                    )

            # ---- recurrence ----
            # state chunk col layout: m * (CHUNK*BS) + t_local * BS + b
            GRP = MC * BS  # 16 state columns per step
            st_views = [s[:].rearrange("p (m t b) -> p m t b", m=MC, b=BS)
                        for s in st_ch]
            for t in range(t_steps):
                ci, tl = divmod(t, CHUNK)
                if t % CHUNK == 0:
                    # observe this H chunk's DVE writes on the PE with a
                    # dummy matmul spanning both m-halves (1 sem wait), so
                    # the MM_I below needs only its psum-slot wait.
                    hspan = h_ch[ci][:].rearrange(
                        "p (t m b) -> p t m b", m=MC, b=BS)[:, 0, :, 0]
                    nc.tensor.matmul(
                        out=junk_ps[0:2, 0:1],
                        lhsT=hspan,
                        rhs=h_ch[ci][:, 0:1],
                        start=True, stop=True,
                    )
                ps = ps_rec.tile([P, GRP], FP32)
                # inject h_t + bias into psum (no dependency on state)
                nc.tensor.matmul(
                    out=ps[:],
                    lhsT=mmap(ident[:]),
                    rhs=mmap(h_ch[ci][:, tl * GRP:(tl + 1) * GRP]),
                    start=True, stop=False,
                )
                for m in range(MC):
                    for c in range(MC):
                        if t == 0:
                            prev_c = state0[:, c * BS:(c + 1) * BS]
                        else:
                            pci, ptl = divmod(t - 1, CHUNK)
                            prev_c = st_views[pci][:, c, ptl, :]
                        nc.tensor.matmul(
                            out=ps[:, m * BS:(m + 1) * BS],
                            lhsT=mmap(w_sb[:, c * U + m * P: c * U + (m + 1) * P]),
                            rhs=mmap(prev_c),
                            start=False, stop=(m == MC - 1 and c == MC - 1),
                        )
                # note: psum cols are (m, b) m-major, matching st layout
                nc.scalar.activation(
                    out=st_views[ci][:, :, tl, :],
                    in_=ps[:].rearrange("p (m b) -> p m b", b=BS),
                    func=mybir.ActivationFunctionType.Tanh,
                )
                # chunk complete -> DMA to DRAM output (one DMA per m).
                # Issued from the ACT engine (qActDynamicHW): the dependency
                # on this chunk's tanh writes is then same-engine program
                # order, so the DMA carries only its queue-FIFO sem wait
                # (walrus allows a single sync wait per instruction).
                if tl == CHUNK - 1:
                    for m in range(MC):
                        nc.scalar.dma_start(
                            out=out_d[ci * CHUNK:(ci + 1) * CHUNK, :,
                                      m * P:(m + 1) * P]
                                .rearrange("t b p -> p t b"),
                            in_=st_views[ci][:, m, :, :],
                        )
    # walrus enforces a single sync wait per instruction. The ACT-issued
    # output DMAs carry (Activation data wait, DMAHW queue-FIFO wait); the
    # queue wait only serializes Tile's virtual DMA lanes — physically the
    # qActDynamicHW ring is FIFO, no mid-kernel consumer waits on these proc
    # sems (only the final drain, which still sees every inc), and the two
    # same-proc DMAs are separated by ~60us of recurrence. Drop it.
    for fn in nc.m.functions:
        for bb in fn.blocks:
            for i in bb.instructions:
                if (type(i).__name__ == "InstDMACopy"
                        and i.engine == mybir.EngineType.Activation):
                    si = i.sync_info
                    if len(si.on_wait) > 1:
                        kept = [w for w in si.on_wait
                                if w.ant_name and "Activation" in w.ant_name]
                        assert kept, f"no ACT wait to keep on {i.name}"
                        si.on_wait = kept

    # Tile's kernel-tail drain carries one wait per outstanding proc (~12),
    # also over the 1-wait limit. Keep its first wait and move the rest onto
    # single-wait SP drains appended at the end of the MAIN block (before the
    # tail barrier and Tile's clear_and_free_semaphores, which zeroes the
    # sems). SP idles there anyway; each drain just holds the kernel open
    # until one proc's final count lands. No cycle: these waits depend only
    # on work other engines have already been unblocked to finish.
    big = None
    main_bb = None
    for fn in nc.m.functions:
        for bb in fn.blocks:
            insts = list(bb.instructions)
            if not bb.name.endswith("_end") and len(insts) > 100:
                main_bb = bb
            for i in insts:
                if (type(i).__name__ == "InstDrain"
                        and i.sync_info is not None
                        and len(i.sync_info.on_wait) > 1):
                    assert big is None, "expected a single multi-wait drain"
                    big = i
    if big is not None:
        assert main_bb is not None
        waits = list(big.sync_info.on_wait)
        big.sync_info.on_wait = waits[:1]
        for k, w in enumerate(waits[1:]):
            d = mybir.InstDrain(name=f"I-tail-drain-{k}", ins=[], outs=[])
            d.engine = mybir.EngineType.SP
            d.sync_info = bass_rust.SyncInfo(on_wait=[w], on_update=[])
            nc.register_instruction(d, overwrite=True)
            main_bb.add_instruction(d)
    return nc


_NC_CACHE: dict = {}


def _get_nc(t_steps: int = T) -> bass.Bass:
    if t_steps not in _NC_CACHE:
        _NC_CACHE[t_steps] = build_nc(t_steps)
    return _NC_CACHE[t_steps]


def make_in_maps(inputs, R, W, bias, x0):
    inputs = np.ascontiguousarray(np.asarray(inputs, dtype=np.float32))
    R = np.ascontiguousarray(np.asarray(R, dtype=np.float32))
    W = np.ascontiguousarray(np.asarray(W, dtype=np.float32))
    bias = np.ascontiguousarray(np.asarray(bias, dtype=np.float32))
    x0 = np.ascontiguousarray(np.asarray(x0, dtype=np.float32))
    return [
        {
            "x": np.ascontiguousarray(inputs[i * BS:(i + 1) * BS]),
            "w": W, "r": R, "bias": bias, "x0": x0,
        }
        for i in range(NCORES)
    ]


def kernel(inputs, R, W, bias, x0):
    nc = _get_nc(T)
    in_maps = make_in_maps(inputs, R, W, bias, x0)
    res = run_bass_kernel_spmd(nc, in_maps, list(range(NCORES))).results
    return np.concatenate([r["out"] for r in res], axis=1)


# revision 28
# speedup vs baseline: 1.6425x; 1.6425x over previous
"""Bass/Tile TRN2 kernel for a tanh RNN layer (BasicRecurrentLayer).

  h = einsum('btf,fu->tbu', inputs, R)
  s_t = tanh(h_t + s_{t-1} @ W + bias);  out[t] = s_t        (T sequential steps)

Shapes: inputs [B=64, T=1024, F=128], R [F, U=256], W [U, U], bias [U], x0 [U].
Output: [T, B, U] float32.

Sharding: data-parallel over batch across 8 cores (8 rows each); R/W/bias/x0
replicated. The T recurrence is inherently serial, so the kernel is built
around minimizing the per-step critical path:

  - transposed state layout sT [U(2x128 part-chunks), B=8] kept entirely in SBUF
  - per step: psum[:,0:16] = I128 @ H_t  (injects h_t+bias, no state dep)
              psum[:, m*8:+8] += W[c,m-block].T @ sT_c   (4 matmuls, W stationary)
              sT_new = tanh(psum)  (single ACT instruction per step)
  - H precomputed in phase 1 (X@R with N=512 matmuls), chunked for pipelining
  - states accumulate in SBUF chunks of 64 steps, DMA'd out in 1MB blocks
"""

import numpy as np

import bass_rust
import concourse.bass as bass
import concourse.mybir as mybir
import concourse.tile as tile
from concourse.bass_utils import run_bass_kernel_spmd
from concourse.masks import make_identity

B, T, F, U = 64, 1024, 128, 256
NCORES = 8
BS = B // NCORES            # 8 batch rows per core
P = 128                     # SBUF partitions
MC = U // P                 # 2 state chunks of 128
CHUNK = 64                  # recurrence steps per state/H chunk buffer
FP32 = mybir.dt.float32


def build_nc(t_steps: int = T, use_f32r: bool = False,
             rec_f32r: bool | None = None) -> bass.Bass:
    """use_f32r: stream phase-1 matmul operands as float32r (same fp32 bits,
    single-pass PE mode, 4x faster at N>=256). rec_f32r: same for the
    recurrence operands (W/H/state); defaults to use_f32r."""
    if rec_f32r is None:
        rec_f32r = use_f32r
    assert t_steps % CHUNK == 0
    n_chunks = t_steps // CHUNK
    nc = bass.Bass()

    # float32r has identical fp32 bits/size (np dtype is float32) but streams
    # through the PE in single-pass reduced-precision mode (~tf32/bf16x2).
    # All producers of float32r buffers must emit float32r (BIR verifier).
    PDT = mybir.dt.float32r if use_f32r else FP32   # phase-1 operands
    MDT = mybir.dt.float32r if rec_f32r else FP32   # recurrence operands

    def mmap(ap):  # matmul-operand view (no-op now; tiles carry dtypes)
        return ap

    x_in = nc.declare_dram_parameter("x", [BS, t_steps, F], PDT, isOutput=False)
    w_in = nc.declare_dram_parameter("w", [U, U], MDT, isOutput=False)
    r_in = nc.declare_dram_parameter("r", [F, U], PDT, isOutput=False)
    b_in = nc.declare_dram_parameter("bias", [U], FP32, isOutput=False)
    x0_in = nc.declare_dram_parameter("x0", [U], FP32, isOutput=False)
    # out is declared with the recurrence dtype label so the state-chunk DMA
    # is cast-free; float32r's numpy dtype is plain float32 either way.
    out_d = nc.declare_dram_parameter("out", [t_steps, BS, U], MDT, isOutput=True)

    with tile.TileContext(nc) as tc:
        with (
            tc.tile_pool(name="sb", bufs=1) as sb,
            tc.tile_pool(name="ps_h", bufs=2, space="PSUM") as ps_h,
            tc.tile_pool(name="ps_rec", bufs=4, space="PSUM") as ps_rec,
        ):
            # ---- persistent SBUF buffers ----
            xt = sb.tile([P, t_steps * BS], MDT, tag="xt")       # X.T, col = t*BS+b
            w_sb = sb.tile([P, MC * U], MDT, tag="w")            # col = c*U + j
            r_sb = sb.tile([P, U], MDT, tag="r")
            ident = sb.tile([P, P], MDT, tag="ident")
            bias_sb = sb.tile([P, MC], FP32, tag="bias")         # col m: bias[m*128+p]
            x0_sb = sb.tile([P, MC], FP32, tag="x0")
            state0 = sb.tile([P, MC * BS], MDT, tag="state0")    # col = m*BS+b
            h_ch = [sb.tile([P, CHUNK * MC * BS], MDT, name=f"h{i}", tag=f"h{i}")
                    for i in range(n_chunks)]
            st_ch = [sb.tile([P, CHUNK * MC * BS], MDT, name=f"s{i}", tag=f"s{i}")
                     for i in range(n_chunks)]

            # ---- constant / input loads ----
            if MDT == FP32:
                make_identity(nc, ident[:])
            else:
                # gpsimd memset/affine_select fail the ISA check on float32r;
                # build the identity in fp32 and cast-copy on the DVE.
                ident_f32 = sb.tile([P, P], FP32, tag="ident_f32")
                make_identity(nc, ident_f32[:])
                nc.vector.tensor_copy(out=ident[:], in_=ident_f32[:])
            for c in range(MC):
                nc.sync.dma_start(out=w_sb[:, c * U:(c + 1) * U],
                                  in_=w_in[c * P:(c + 1) * P, :])
            nc.sync.dma_start(out=r_sb[:], in_=r_in[:])
            nc.sync.dma_start(out=bias_sb[:],
                              in_=b_in[:].rearrange("(m p) -> p m", m=MC))
            nc.sync.dma_start(out=x0_sb[:],
                              in_=x0_in[:].rearrange("(m p) -> p m", m=MC))
            for m in range(MC):
                nc.vector.tensor_copy(
                    out=state0[:, m * BS:(m + 1) * BS],
                    in_=x0_sb[:, m:m + 1].to_broadcast([P, BS]),
                )
            # observe the bias DMA's queue sem on the DVE (1 wait here) so
            # phase-1 tensor_scalar_adds only carry their psum (PE) wait —
            # walrus allows a single sync wait per DVE instruction.
            scratch = sb.tile([P, MC], FP32, tag="scratch")
            nc.vector.tensor_copy(out=scratch[:], in_=bias_sb[:])

            # inputs -> X.T in SBUF, one DMA per batch row (2D APs each side)
            xt_v = xt[:].rearrange("p (t b) -> p t b", b=BS)
            for b in range(BS):
                nc.sync.dma_start(
                    out=xt_v[:, :, b],
                    in_=x_in[b, :, :].rearrange("t f -> f t"),
                )

            # ---- PE observation ladder ----
            # walrus can attach only ONE sem wait to a Matmult (fp32
            # self-loading LDWEIGHTS wait table). The 8 input DMAs land on 8
            # different HW-DGE queue sems, so a phase-1 matmul reading XT
            # would need up to 8 waits. These dummy matmuls observe one
            # queue sem each so later PE instructions need no input waits.
            junk_ps = ps_h.tile([P, 2], FP32, tag="junk", bufs=1)
            for k in range(BS):
                nc.tensor.matmul(
                    out=junk_ps[0:1, 0:1],
                    lhsT=xt[:, k:k + 1],
                    rhs=xt[:, k:k + 1],
                    start=True, stop=True,
                )

            # ---- phase 1: H.T = R.T @ X.T (+bias), chunked to pipeline ----
            # psum [128, CHUNK*BS] per (chunk, m); cols (t_local, b)
            for i in range(n_chunks):
                hv = h_ch[i][:].rearrange("p (t m b) -> p t m b", m=MC, b=BS)
                for m in range(MC):
                    if i > 0:
                        # the psum slot for (i, m) was last read by the DVE
                        # copy of (i-1, m); observe that DVE tick with a
                        # 1-wait dummy so the real matmul below only carries
                        # its PE (prior-matmul completion) wait.
                        nc.tensor.matmul(
                            out=junk_ps[0:1, 0:1],
                            lhsT=h_ch[i - 1][:, m * BS:m * BS + 1],
                            rhs=h_ch[i - 1][:, m * BS:m * BS + 1],
                            start=True, stop=True,
                        )
                    ph = ps_h.tile([P, CHUNK * BS], FP32)
                    nc.tensor.matmul(
                        out=ph[:],
                        lhsT=mmap(r_sb[:, m * P:(m + 1) * P]),
                        rhs=mmap(xt[:, i * CHUNK * BS:(i + 1) * CHUNK * BS]),
                        start=True, stop=True,
                    )
                    nc.vector.tensor_scalar_add(
                        out=hv[:, :, m, :],
                        in0=ph[:].rearrange("p (t b) -> p t b", b=BS),
                        scalar1=bias_sb[:, m:m + 1],
                    )

            # ---- recurrence ----
            # state chunk col layout: m * (CHUNK*BS) + t_local * BS + b
            GRP = MC * BS  # 16 state columns per step
            st_views = [s[:].rearrange("p (m t b) -> p m t b", m=MC, b=BS)
                        for s in st_ch]
            for t in range(t_steps):
                ci, tl = divmod(t, CHUNK)
                if t % CHUNK == 0:
                    # observe this H chunk's DVE writes on the PE with a
                    # dummy matmul spanning both m-halves (1 sem wait), so
                    # the MM_I below needs only its psum-slot wait.
                    hspan = h_ch[ci][:].rearrange(
                        "p (t m b) -> p t m b", m=MC, b=BS)[:, 0, :, 0]
                    nc.tensor.matmul(
                        out=junk_ps[0:2, 0:1],
                        lhsT=hspan,
                        rhs=h_ch[ci][:, 0:1],
                        start=True, stop=True,
                    )
                ps = ps_rec.tile([P, GRP], FP32)
                # inject h_t + bias into psum (no dependency on state)
                nc.tensor.matmul(
                    out=ps[:],
                    lhsT=mmap(ident[:]),
                    rhs=mmap(h_ch[ci][:, tl * GRP:(tl + 1) * GRP]),
                    start=True, stop=False,
                )
                for m in range(MC):
                    for c in range(MC):
                        if t == 0:
                            prev_c = state0[:, c * BS:(c + 1) * BS]
                        else:
                            pci, ptl = divmod(t - 1, CHUNK)
                            prev_c = st_views[pci][:, c, ptl, :]
                        nc.tensor.matmul(
                            out=ps[:, m * BS:(m + 1) * BS],
                            lhsT=mmap(w_sb[:, c * U + m * P: c * U + (m + 1) * P]),
                            rhs=mmap(prev_c),
                            start=False, stop=(m == MC - 1 and c == MC - 1),
                        )
                # note: psum cols are (m, b) m-major, matching st layout
                nc.scalar.activation(
                    out=st_views[ci][:, :, tl, :],
                    in_=ps[:].rearrange("p (m b) -> p m b", b=BS),
                    func=mybir.ActivationFunctionType.Tanh,
                )
                # chunk complete -> DMA to DRAM output (one DMA per m).
                # sync-engine HWDGE; the post-pass below drops the queue-FIFO
                # wait (keeping the ACT data wait) to fit walrus's 1-wait
                # limit. ACT-ring issue was tried and blocks the ACT engine
                # ~210us per DMA ("queue 10 invalid" runtime fallback).
                if tl == CHUNK - 1:
                    for m in range(MC):
                        nc.sync.dma_start(
                            out=out_d[ci * CHUNK:(ci + 1) * CHUNK, :,
                                      m * P:(m + 1) * P]
                                .rearrange("t b p -> p t b"),
                            in_=st_views[ci][:, m, :, :],
                        )
    # walrus enforces a single sync wait per instruction. The output DMAs
    # carry (Activation data wait, DMAHW queue-FIFO wait). The queue wait
    # only serializes Tile's virtual DMA lanes: no mid-kernel consumer waits
    # on the output DMAs' proc sems (only the final drain, which still sees
    # every inc), and same-proc predecessors finished ~100us earlier, so
    # dropping it is safe. The ACT data wait (the real dependency) stays.
    for fn in nc.m.functions:
        for bb in fn.blocks:
            for i in bb.instructions:
                if (type(i).__name__ == "InstDMACopy"
                        and i.sync_info is not None
                        and len(i.sync_info.on_wait) > 1):
                    si = i.sync_info
                    kept = [w for w in si.on_wait
                            if w.ant_name and "Activation" in w.ant_name]
                    assert kept, f"no ACT wait to keep on {i.name}"
                    si.on_wait = kept

    # Tile's kernel-tail drain carries one wait per outstanding proc (~12),
    # also over the 1-wait limit. Keep its first wait and move the rest onto
    # single-wait SP drains appended at the end of the MAIN block (before the
    # tail barrier and Tile's clear_and_free_semaphores, which zeroes the
    # sems). SP idles there anyway; each drain just holds the kernel open
    # until one proc's final count lands. No cycle: these waits depend only
    # on work other engines have already been unblocked to finish.
    big = None
    main_bb = None
    for fn in nc.m.functions:
        for bb in fn.blocks:
            insts = list(bb.instructions)
            if not bb.name.endswith("_end") and len(insts) > 100:
                main_bb = bb
            for i in insts:
                if (type(i).__name__ == "InstDrain"
                        and i.sync_info is not None
                        and len(i.sync_info.on_wait) > 1):
                    assert big is None, "expected a single multi-wait drain"
                    big = i
    if big is not None:
        assert main_bb is not None
        waits = list(big.sync_info.on_wait)
        big.sync_info.on_wait = waits[:1]
        for k, w in enumerate(waits[1:]):
            d = mybir.InstDrain(name=f"I-tail-drain-{k}", ins=[], outs=[])
            d.engine = mybir.EngineType.SP
            d.sync_info = bass_rust.SyncInfo(on_wait=[w], on_update=[])
            nc.register_instruction(d, overwrite=True)
            main_bb.add_instruction(d)
    return nc


_NC_CACHE: dict = {}


def _get_nc(t_steps: int = T) -> bass.Bass:
    if t_steps not in _NC_CACHE:
        _NC_CACHE[t_steps] = build_nc(t_steps)
    return _NC_CACHE[t_steps]


def make_in_maps(inputs, R, W, bias, x0):
    inputs = np.ascontiguousarray(np.asarray(inputs, dtype=np.float32))
    R = np.ascontiguousarray(np.asarray(R, dtype=np.float32))
    W = np.ascontiguousarray(np.asarray(W, dtype=np.float32))
    bias = np.ascontiguousarray(np.asarray(bias, dtype=np.float32))
    x0 = np.ascontiguousarray(np.asarray(x0, dtype=np.float32))
    return [
        {
            "x": np.ascontiguousarray(inputs[i * BS:(i + 1) * BS]),
            "w": W, "r": R, "bias": bias, "x0": x0,
        }
        for i in range(NCORES)
    ]


def kernel(inputs, R, W, bias, x0):
    nc = _get_nc(T)
    in_maps = make_in_maps(inputs, R, W, bias, x0)
    res = run_bass_kernel_spmd(nc, in_maps, list(range(NCORES))).results
    return np.concatenate([r["out"] for r in res], axis=1)
